# revision 6
# baseline (speedup 1.0000x reference)
"""MinibatchDiscrimination kernel for Trainium2 (8 NeuronCores, SPMD).

Problem:  x [256, 1024] f32, T [1024, 128, 32] f32
          M = einsum('ni,iok->nok', x, T)
          norm[a,b,o] = sum_k |M[a,o,k] - M[b,o,k]|
          o_b = exp(-norm).sum(axis=0) - 1            # [256, 128]
          out = concat([x, o_b], axis=1)              # [256, 1152]

Sharding: data-parallel over out_features of T — each of the 8 cores
computes the pairwise reduction for its 16 output channels; x replicated.

Per-core dataflow (v6):

  MT[(o,k), a] = Tsh^T @ x^T           PE, 4 chunks of [128, 256] bf16
                 partition p = o*8 + k_l, chunk g holds k = 8g + k_l
  ST[o, a]     = sum_k MT              PE (ones stationary), bf16

  |d| = 2*relu(d) - d and sum_k d_k = S_a - S_b (rank-1), so a norm tile
  accumulates 2*relu(d) matmuls, one -S_a matmul (negsa8, emitted last),
  and -S_b as the exp bias.

  Pairwise in 8 triangle blocks of 32 b's (block i covers a >= 32i; the
  a < 32i region is recovered from transposed E tiles of earlier blocks).
  GROUP PAIRING: two groups of 8 b's (b = 8*grp + l) share one PSUM tile
  nt2 [16*l + o, 2*FD]; a DVE slot's ones-matmul then carries both
  groups' relu tiles in one instruction (halves the PE instruction count).

  Relu-tile engine split per slot (= 2 chunks of one b, both groups):
    D: DVE ts(subtract, max 0) bf16 (4x mode), packed [128, 2(b), FD]
       -> 1 bf16 matmul per chunk covering both groups
    P: GPSIMD ts, fp8e5, packed [128, 2(chunk), FD] -> fp8 DoubleRow
       matmul per group (2 k-chunks per pass)
    A: ACT activation(Relu, bias=-M[:,b]) fp8e5 -> DoubleRow per group

  exp:  ACT activation(Exp, scale=-1, bias=-S_b) + accum_out col sums
  obT:  per block, PE matmul (sel8 stationary) of E[:, a >= 32(i+1)] sums
        the 8 b-bands; DVE accumulates into SBUF across blocks.

Numerics: norms are O(100..4000) and reach the output only through
exp(-norm) which underflows to 0; fp8e5 relu tiles (rel err ~6%/elem,
range 57344 so no overflow) cannot move any norm below the exp(-20)
visibility floor.
"""

import os as _os_mod
# The axon NTFF profile hook module is absent in this environment; if the
# caller's env has BASS_TRACE set, run_bass_kernel_spmd would crash trying
# to import it.  Force the no-trace path.
_os_mod.environ["BASS_NEVER_TRACE"] = "1"

import numpy as np
import ml_dtypes

import concourse.bass as bass
import concourse.bacc as bacc
import concourse.mybir as mybir
import concourse.tile as tile
from concourse.bass_utils import run_bass_kernel_spmd

BF16 = ml_dtypes.bfloat16
F8E5 = ml_dtypes.float8_e5m2

N = 256          # batch
IN_F = 1024      # in features
OUT_F = 128      # out features (total)
K = 32           # kernel dim
NCORES = 8
O = OUT_F // NCORES   # out features per core (16)
NBLK = 8              # triangle blocks
BW = N // NBLK        # b's per block (32)
NGRP = 32             # groups of 8 b's
GL = 8                # b's per group

# Per-block engine schedule: 16 chars (slot = 2*l + pair) from {D, P, A}.
# Slot 0 must be D (it opens the PSUM accumulation over the full pair).
DEFAULT_SCHED = [
    "DPDADDPDADDPDADD",   # FD=256
    "DPDADDPDADDPADDD",   # FD=224
    "DPDADDPDADDPADDD",   # FD=192
    "DPDADDPDDDPPADDD",   # FD=160
    "DPDADDPDDDDPADDD",   # FD=128
    "DPDADPDDDPDADPDD",   # FD=96
    "DPDADPDDDPDADPDD",   # FD=64
    "DPDADPDDDPDADPDD",   # FD=32
]


def _sched():
    s = _os_mod.environ.get("V5_SCHED", "")
    if s:
        blocks = s.split(",")
        assert len(blocks) == NBLK and all(len(b) == 16 for b in blocks)
    else:
        blocks = DEFAULT_SCHED
    assert all(b[0] == "D" for b in blocks)
    return blocks


def build_core_program(reps=1, **_legacy):
    sched = _sched()
    nc = bacc.Bacc("TRN2", target_bir_lowering=False)

    # xt[p, it, a] = x^T[it*128 + p, a]; tsh[p, g, it, m] = Tsh[it*128 + p, g*128 + m]
    xt_d = nc.dram_tensor("xt", [128, 8, N], mybir.dt.bfloat16, kind="ExternalInput")
    tsh_d = nc.dram_tensor("tsh", [128, 4, 8, 128], mybir.dt.bfloat16, kind="ExternalInput")
    # bf16 consts on 128 partitions: 8 ones(2.0) [128,128], sel8 [128,16], bones1 [128,16]
    cbf_d = nc.dram_tensor("cbf", [128, 8 * 128 + 32], mybir.dt.bfloat16, kind="ExternalInput")
    # bf16 consts on 16 partitions: negsa8 [16,128], negselb8[l] [16,128] x8
    csm_d = nc.dram_tensor("csm", [16, 9 * 128], mybir.dt.bfloat16, kind="ExternalInput")
    # fp8 DoubleRow ones(2.0) stationaries [128, 2, 128] per l
    cdr_d = nc.dram_tensor("cdr", [128, 8, 2, 128], mybir.dt.float8e5, kind="ExternalInput")
    ob_d = nc.dram_tensor("ob", [128, NGRP], mybir.dt.float32, kind="ExternalOutput")
    ob2_d = nc.dram_tensor("ob2", [O, N - BW], mybir.dt.float32, kind="ExternalOutput")

    import os as _os
    AD_BUFS = int(_os.environ.get("AD_BUFS", "16"))
    AD8_BUFS = int(_os.environ.get("AD8_BUFS", "12"))
    E_BUFS = int(_os.environ.get("E_BUFS", "8"))
    PNORM_BUFS = int(_os.environ.get("PNORM_BUFS", "6"))
    OBT_BUFS = int(_os.environ.get("OBT_BUFS", "2"))
    INTERLEAVE = _os.environ.get("V6_INTERLEAVE", "1") == "1"

    with tile.TileContext(nc) as tc:
        with (
            tc.tile_pool(name="weights", bufs=1) as wpool,
            tc.tile_pool(name="mt", bufs=1) as mtpool,
            tc.tile_pool(name="absd", bufs=AD_BUFS) as adpool,
            tc.tile_pool(name="absd8", bufs=AD8_BUFS) as ad8pool,
            tc.tile_pool(name="escratch", bufs=E_BUFS) as epool,
            tc.tile_pool(name="obp", bufs=1) as obpool,
        ):
            setup_psum_cm = tc.tile_pool(name="psum_mt", bufs=2, space=bass.MemorySpace.PSUM)
            pmt = setup_psum_cm.__enter__()
            psmall_cm = tc.tile_pool(name="psum_s", bufs=1, space=bass.MemorySpace.PSUM)
            psmall = psmall_cm.__enter__()

            # ---- load inputs (few big DMAs; tsh per-chunk so MT can start early) ----
            xt_t = wpool.tile([128, 8, N], mybir.dt.bfloat16, tag="xt")
            nc.sync.dma_start(xt_t[:], xt_d[:])
            tsh_t = wpool.tile([128, 4, 8, 128], mybir.dt.bfloat16, tag="tsh")
            for g in range(4):
                nc.sync.dma_start(tsh_t[:, g, :, :], tsh_d[:, g, :, :])
            cbf = wpool.tile([128, 8 * 128 + 32], mybir.dt.bfloat16)
            nc.sync.dma_start(cbf[:], cbf_d[:])
            w2bf = [cbf[:, 128 * l:128 * (l + 1)] for l in range(GL)]
            sel8 = cbf[:, 1024:1040]
            bones1 = cbf[:, 1040:1056]
            cdr = wpool.tile([128, 8, 2, 128], mybir.dt.float8e5, tag="cdr")
            nc.sync.dma_start(cdr[:], cdr_d[:])
            wdr = [cdr[:, l, :, :] for l in range(GL)]
            csm = wpool.tile([16, 9 * 128], mybir.dt.bfloat16, tag="csm")
            nc.sync.dma_start(csm[:], csm_d[:])
            negsa8 = csm[:, 0:128]
            negselb8 = [csm[:, 128 * (1 + l):128 * (2 + l)] for l in range(GL)]

            # ---- MT = Tsh^T @ x^T : [(o,k), a] in 4 chunks of 128 partitions ----
            mt = []      # bf16 working copy
            mtf32 = []   # fp32 upcast of the bf16-rounded values (ts scalar operand)
            nmt32 = []   # negated fp32 (ACT Relu bias)
            for g in range(4):
                pm = pmt.tile([128, N], mybir.dt.float32)
                for it in range(8):
                    nc.tensor.matmul(
                        pm[:],
                        tsh_t[:, g, it, :],
                        xt_t[:, it, :],
                        start=(it == 0),
                        stop=(it == 7),
                    )
                mt_g = mtpool.tile([128, N], mybir.dt.bfloat16, tag=f"mt{g}")
                nc.vector.tensor_copy(mt_g[:], pm[:])
                mt32_g = mtpool.tile([128, N], mybir.dt.float32, tag=f"mt32{g}")
                nc.scalar.copy(mt32_g[:], mt_g[:])
                nm_g = mtpool.tile([128, N], mybir.dt.float32, tag=f"nmt32{g}")
                nc.gpsimd.tensor_scalar(
                    nm_g[:], mt_g[:], -1.0, None, mybir.AluOpType.mult,
                )
                mt.append(mt_g)
                mtf32.append(mt32_g)
                nmt32.append(nm_g)

            # ---- ST[o, a] = sum_k MT ----
            st_ps = psmall.tile([16, N], mybir.dt.float32, tag="st_ps")
            for g in range(4):
                nc.tensor.matmul(
                    st_ps[:], bones1, mt[g][:], start=(g == 0), stop=(g == 3)
                )
            st_bf = mtpool.tile([16, N], mybir.dt.bfloat16, tag="st_bf")
            nc.vector.tensor_copy(st_bf[:], st_ps[:])

            # ---- bias tile: negsb8[16*l + o, grp] = -ST[o, 8*grp + l] ----
            nsb_ps = psmall.tile([128, NGRP], mybir.dt.float32, tag="nsb_ps")
            for l in range(GL):
                nc.tensor.matmul(
                    nsb_ps[:], negselb8[l], st_bf[:, l::GL],
                    start=(l == 0), stop=(l == GL - 1),
                )
            negsb8 = obpool.tile([128, NGRP], mybir.dt.float32, tag="negsb8")
            nc.vector.tensor_copy(negsb8[:], nsb_ps[:])

            ob_acc = obpool.tile([128, NGRP], mybir.dt.float32)
            obt_acc = obpool.tile([O, N - BW], mybir.dt.float32, tag="obt_acc")
            nc.vector.memset(obt_acc[:], 0.0)

            psmall_cm.__exit__(None, None, None)
            setup_psum_cm.__exit__(None, None, None)
            pnorm_cm = tc.tile_pool(
                name="psum_norm", bufs=PNORM_BUFS, space=bass.MemorySpace.PSUM,
            )
            pnorm = pnorm_cm.__enter__()
            obt_cm = tc.tile_pool(name="psum_obt", bufs=OBT_BUFS, space=bass.MemorySpace.PSUM)
            obt_pool = obt_cm.__enter__()

            # ---- pairwise: 8 triangle blocks x 2 group-pairs of 2x8 b's ----
            # Pair emission order: optionally interleave big-FD and small-FD
            # blocks so the latency-bound tail overlaps dense work.
            pair_order = [(blk, jp) for blk in range(NBLK) for jp in range(2)]
            if INTERLEAVE:
                first = [(blk, jp) for blk in range(NBLK // 2) for jp in range(2)]
                second = [(blk, jp) for blk in range(NBLK // 2, NBLK) for jp in range(2)][::-1]
                pair_order = [x for p in zip(first, second) for x in p]

            obt_state = {}   # blk -> (obt_ps tile, pairs_done)

            import contextlib
            rep_ctx = tc.For_i(0, reps, 1) if reps > 1 else contextlib.nullcontext()
            with rep_ctx:
                for blk, jp in pair_order:
                    a0 = BW * blk
                    FD = N - a0
                    pat = sched[blk]
                    if blk not in obt_state:
                        obt_ps = None
                        if blk < NBLK - 1:
                            obt_ps = obt_pool.tile([O, FD - BW], mybir.dt.float32, tag="obt")
                        obt_state[blk] = [obt_ps, 0]
                    obt_ps = obt_state[blk][0]

                    grp0 = 4 * blk + 2 * jp          # first group of the pair
                    bb = [a0 + GL * (2 * jp + h) for h in range(2)]  # b base per half
                    nt2 = pnorm.tile([128, 2 * FD], mybir.dt.float32, tag="nt")
                    first_mm = True
                    for l in range(GL):
                        for pair in range(2):
                            eng = pat[2 * l + pair]
                            g0, g1 = 2 * pair, 2 * pair + 1
                            if eng == "D":
                                # one [128, 2(b), FD] tile per chunk, both groups
                                for g in (g0, g1):
                                    ad = adpool.tile([128, 2, N], mybir.dt.bfloat16, tag="adD")
                                    for h in range(2):
                                        nc.vector.tensor_scalar(
                                            ad[:, h, :FD], mt[g][:, a0:],
                                            mtf32[g][:, bb[h] + l:bb[h] + l + 1], 0.0,
                                            mybir.AluOpType.subtract, mybir.AluOpType.max,
                                        )
                                    nc.tensor.matmul(
                                        nt2[:, 0:2 * FD], w2bf[l], ad[:, :, :FD],
                                        start=first_mm, stop=False,
                                        skip_group_check=True,
                                    )
                                    first_mm = False
                            else:
                                for h in range(2):
                                    b = bb[h] + l
                                    ad2 = ad8pool.tile([128, 2, N], mybir.dt.float8e5, tag="ad8")
                                    if eng == "P":
                                        for i, g in enumerate((g0, g1)):
                                            nc.gpsimd.tensor_scalar(
                                                ad2[:, i, :FD], mt[g][:, a0:],
                                                mtf32[g][:, b:b + 1], 0.0,
                                                mybir.AluOpType.subtract, mybir.AluOpType.max,
                                            )
                                    else:
                                        for i, g in enumerate((g0, g1)):
                                            nc.scalar.activation(
                                                ad2[:, i, :FD], mt[g][:, a0:],
                                                mybir.ActivationFunctionType.Relu,
                                                bias=nmt32[g][:, b:b + 1],
                                            )
                                    nc.tensor.matmul(
                                        nt2[:, h * FD:(h + 1) * FD], wdr[l], ad2[:, :, :FD],
                                        start=False, stop=False,
                                        perf_mode=mybir.MatmulPerfMode.DoubleRow,
                                        skip_group_check=True,
                                    )
                    # -S_a terms close each half's accumulation
                    for h in range(2):
                        nc.tensor.matmul(
                            nt2[:, h * FD:(h + 1) * FD], negsa8, st_bf[:, a0:],
                            start=False, stop=True, skip_group_check=True,
                        )
                    for h in range(2):
                        grp = grp0 + h
                        e = epool.tile([128, N], mybir.dt.bfloat16, tag="e")
                        nc.scalar.activation(
                            e[:, :FD], nt2[:, h * FD:(h + 1) * FD],
                            mybir.ActivationFunctionType.Exp,
                            scale=-1.0, bias=negsb8[:, grp:grp + 1],
                            accum_out=ob_acc[:, grp:grp + 1],
                        )
                        if obt_ps is not None:
                            nc.tensor.matmul(
                                obt_ps[:], sel8, e[:, BW:FD],
                                start=(obt_state[blk][1] == 0 and h == 0),
                                stop=(obt_state[blk][1] == 1 and h == 1),
                                skip_group_check=True,
                            )
                    obt_state[blk][1] += 1
                    if obt_state[blk][1] == 2 and obt_ps is not None:
                        nc.vector.tensor_tensor(
                            obt_acc[:, a0:], obt_acc[:, a0:], obt_ps[:],
                            mybir.AluOpType.add,
                        )
                    if obt_state[blk][1] == 2:
                        del obt_state[blk]

            obt_cm.__exit__(None, None, None)
            pnorm_cm.__exit__(None, None, None)
            ob_final = obpool.tile([128, NGRP], mybir.dt.float32)
            nc.vector.tensor_scalar_add(ob_final[:], ob_acc[:], -1.0)
            nc.sync.dma_start(ob_d[:], ob_final[:])
            nc.sync.dma_start(ob2_d[:], obt_acc[:])

    nc.compile()
    return nc


def host_prep_shared(x):
    xt = np.ascontiguousarray(
        x.T.reshape(8, 128, N).transpose(1, 0, 2)
    ).astype(BF16)                                       # [128, 8, 256]
    cbf = np.zeros((128, 8 * 128 + 32), dtype=BF16)
    for l in range(GL):
        for p in range(128):
            cbf[p, 128 * l + 16 * l + p // 8] = 2.0      # w2bf[l]
    for p in range(128):
        cbf[p, 1024 + (p % 16)] = 1.0                    # sel8
        cbf[p, 1040 + p // 8] = 1.0                      # bones1
    csm = np.zeros((16, 9 * 128), dtype=BF16)
    for o in range(16):
        for l in range(GL):
            csm[o, 16 * l + o] = -1.0                    # negsa8
            csm[o, 128 * (1 + l) + 16 * l + o] = -1.0    # negselb8[l]
    cdr = np.zeros((128, 8, 2, 128), dtype=F8E5)
    for l in range(GL):
        for p in range(128):
            cdr[p, l, :, 16 * l + p // 8] = 2.0          # wdr[l]
    return xt, cbf, csm, cdr


def pack_tsh(T_core):
    """T_core [IN_F, O, K] -> [128, 4, 8, 128]: [p, g, it, m] = Tsh[it*128+p, g*128+m]
    with Tsh col m = o*8 + k_l, k = 8g + k_l."""
    tsh = np.ascontiguousarray(
        T_core.reshape(IN_F, O, 4, 8).transpose(0, 2, 1, 3).reshape(IN_F, 4, 128)
    )                                                    # [i, g, m]
    return np.ascontiguousarray(
        tsh.reshape(8, 128, 4, 128).transpose(1, 2, 0, 3)
    ).astype(BF16)                                       # [p, g, it, m]


def make_in_maps(x, T):
    xt, cbf, csm, cdr = host_prep_shared(x)
    in_maps = []
    for c in range(NCORES):
        tsh = pack_tsh(T[:, c * O:(c + 1) * O, :])
        in_maps.append({"xt": xt, "tsh": tsh, "cbf": cbf, "csm": csm, "cdr": cdr})
    return in_maps


def unscramble(ob_raw, ob2):
    """ob_raw [128, 32], ob2 [16, 224] -> ob [256, 16].

    b = 8*grp + l; ob_raw row = 16*l + o, col = grp.
    ob2[o, a'-32] holds the transposed-triangle contributions for a' >= 32.
    """
    a = np.asarray(ob_raw).reshape(GL, O, NGRP)        # [l, o, grp]
    ob = a.transpose(2, 0, 1).reshape(N, O).copy()     # [b, o]
    ob[BW:, :] += np.asarray(ob2).T
    return ob


_NC_CACHE = None


def kernel(x, T):
    global _NC_CACHE
    x = np.asarray(x, dtype=np.float32)
    T = np.asarray(T, dtype=np.float32)
    assert x.shape == (N, IN_F) and T.shape == (IN_F, OUT_F, K)

    if _NC_CACHE is None:
        _NC_CACHE = build_core_program()
    nc = _NC_CACHE

    in_maps = make_in_maps(x, T)
    res = run_bass_kernel_spmd(nc, in_maps, core_ids=list(range(NCORES)))

    cores = [unscramble(r["ob"], r["ob2"]) for r in res.results]
    ob = np.concatenate(cores, axis=1).astype(np.float32)

    out = np.empty((N, IN_F + OUT_F), dtype=np.float32)
    out[:, :IN_F] = x
    out[:, IN_F:] = ob
    return out


# revision 16
# speedup vs baseline: 3.9025x; 3.9025x over previous
"""MinibatchDiscrimination kernel for Trainium2 (8 NeuronCores, SPMD).

Problem:  x [256, 1024] f32, T [1024, 128, 32] f32
          M = einsum('ni,iok->nok', x, T)
          norm[a,b,o] = sum_k |M[a,o,k] - M[b,o,k]|
          o_b = exp(-norm).sum(axis=0) - 1            # [256, 128]
          out = concat([x, o_b], axis=1)              # [256, 1152]

Sharding: data-parallel over out_features of T — each of the 8 cores
computes the pairwise reduction for its 16 output channels; x replicated.

Per-core dataflow (v6):

  MT[(o,k), a] = Tsh^T @ x^T           PE, 4 chunks of [128, 256] bf16
                 partition p = o*8 + k_l, chunk g holds k = 8g + k_l
  ST[o, a]     = sum_k MT              PE (ones stationary), bf16

  |d| = 2*relu(d) - d and sum_k d_k = S_a - S_b (rank-1), so a norm tile
  accumulates 2*relu(d) matmuls, one -S_a matmul (negsa8, emitted last),
  and -S_b as the exp bias.

  Pairwise in 8 triangle blocks of 32 b's (block i covers a >= 32i; the
  a < 32i region is recovered from transposed E tiles of earlier blocks).
  GROUP PAIRING: two groups of 8 b's (b = 8*grp + l) share one PSUM tile
  nt2 [16*l + o, 2*FD]; a DVE slot's ones-matmul then carries both
  groups' relu tiles in one instruction (halves the PE instruction count).

  Relu-tile engine split per slot (= 2 chunks of one b, both groups):
    D: DVE ts(subtract, max 0) bf16 (4x mode), packed [128, 2(b), FD]
       -> 1 bf16 matmul per chunk covering both groups
    P: GPSIMD ts, fp8e5, packed [128, 2(chunk), FD] -> fp8 DoubleRow
       matmul per group (2 k-chunks per pass)
    A: ACT activation(Relu, bias=-M[:,b]) fp8e5 -> DoubleRow per group

  exp:  ACT activation(Exp, scale=-1, bias=-S_b) + accum_out col sums
  obT:  per block, PE matmul (sel8 stationary) of E[:, a >= 32(i+1)] sums
        the 8 b-bands; DVE accumulates into SBUF across blocks.

Numerics: norms are O(100..4000) and reach the output only through
exp(-norm) which underflows to 0; fp8e5 relu tiles (rel err ~6%/elem,
range 57344 so no overflow) cannot move any norm below the exp(-20)
visibility floor.
"""

import os as _os_mod
# The axon NTFF profile hook module is absent in this environment; if the
# caller's env has BASS_TRACE set, run_bass_kernel_spmd would crash trying
# to import it.  Force the no-trace path.
_os_mod.environ["BASS_NEVER_TRACE"] = "1"

import numpy as np
import ml_dtypes

import concourse.bass as bass
import concourse.bacc as bacc
import concourse.mybir as mybir
import concourse.tile as tile
from concourse.bass_utils import run_bass_kernel_spmd

BF16 = ml_dtypes.bfloat16
F8E5 = ml_dtypes.float8_e5m2

N = 256          # batch
IN_F = 1024      # in features
OUT_F = 128      # out features (total)
K = 32           # kernel dim
NCORES = 8
O = OUT_F // NCORES   # out features per core (16)
NBLK = 8              # triangle blocks
BW = N // NBLK        # b's per block (32)
NGRP = 32             # groups of 8 b's
GL = 8                # b's per group

# Per-block engine schedule: 16 chars (slot = 2*l + pair) from {D, P, A}.
# Slot 0 must be D (it opens the PSUM accumulation over the full pair).
# HW-tuned: GPSIMD turned out ~8x slower than the cost model (3.8us per
# tile) so no P slots; DVE carries 12/16 and ACT 4/16 of the relu tiles,
# which balances the two engines' per-instruction fixed costs.
DEFAULT_SCHED = ["DADDDADDDADDDADD"] * 8


def _sched():
    s = _os_mod.environ.get("V5_SCHED", "")
    if s:
        blocks = s.split(",")
        assert len(blocks) == NBLK and all(len(b) == 16 for b in blocks)
    else:
        blocks = DEFAULT_SCHED
    assert all(b[0] == "D" for b in blocks)
    return blocks


def build_core_program(reps=1, **_legacy):
    sched = _sched()
    nc = bacc.Bacc("TRN2", target_bir_lowering=False)

    # xt[p, it, a] = x^T[it*128 + p, a]; tsh[p, g, it, m] = Tsh[it*128 + p, g*128 + m]
    xt_d = nc.dram_tensor("xt", [128, 8, N], mybir.dt.bfloat16, kind="ExternalInput")
    tsh_d = nc.dram_tensor("tsh", [128, 4, 8, 128], mybir.dt.bfloat16, kind="ExternalInput")
    # bf16 consts on 128 partitions: 8 ones(2.0) [128,128], sel8 [128,16], bones1 [128,16]
    cbf_d = nc.dram_tensor("cbf", [128, 8 * 128 + 32], mybir.dt.bfloat16, kind="ExternalInput")
    # bf16 consts on 16 partitions: negsa8 [16,128], negselb8[l] [16,128] x8
    csm_d = nc.dram_tensor("csm", [16, 9 * 128], mybir.dt.bfloat16, kind="ExternalInput")
    # fp8 DoubleRow ones(2.0) stationaries [128, 2, 128] per l
    cdr_d = nc.dram_tensor("cdr", [128, 8, 2, 128], mybir.dt.float8e5, kind="ExternalInput")
    ob_d = nc.dram_tensor("ob", [128, NGRP], mybir.dt.float32, kind="ExternalOutput")
    ob2_d = nc.dram_tensor("ob2", [O, N - BW], mybir.dt.float32, kind="ExternalOutput")

    import os as _os
    AD_BUFS = int(_os.environ.get("AD_BUFS", "16"))
    AD8_BUFS = int(_os.environ.get("AD8_BUFS", "12"))
    E_BUFS = int(_os.environ.get("E_BUFS", "8"))
    PNORM_BUFS = int(_os.environ.get("PNORM_BUFS", "6"))
    OBT_BUFS = int(_os.environ.get("OBT_BUFS", "2"))
    INTERLEAVE = _os.environ.get("V6_INTERLEAVE", "0") == "1"
    PAIR_MM = _os.environ.get("V6_PAIR_MM", "1") == "1"
    ABLATE = _os.environ.get("V6_ABLATE", "")   # "", "nomm", "nots"
    SPLIT_TS = _os.environ.get("V6_SPLIT_TS", "0") == "1"

    with tile.TileContext(nc) as tc:
        with (
            tc.tile_pool(name="weights", bufs=1) as wpool,
            tc.tile_pool(name="mt", bufs=1) as mtpool,
            tc.tile_pool(name="absd", bufs=AD_BUFS) as adpool,
            tc.tile_pool(name="absd8", bufs=AD8_BUFS) as ad8pool,
            tc.tile_pool(name="escratch", bufs=E_BUFS) as epool,
            tc.tile_pool(name="obp", bufs=1) as obpool,
        ):
            setup_psum_cm = tc.tile_pool(name="psum_mt", bufs=2, space=bass.MemorySpace.PSUM)
            pmt = setup_psum_cm.__enter__()
            psmall_cm = tc.tile_pool(name="psum_s", bufs=1, space=bass.MemorySpace.PSUM)
            psmall = psmall_cm.__enter__()

            # ---- load inputs (few big DMAs; tsh per-chunk so MT can start early) ----
            xt_t = wpool.tile([128, 8, N], mybir.dt.bfloat16, tag="xt")
            nc.sync.dma_start(xt_t[:], xt_d[:])
            tsh_t = wpool.tile([128, 4, 8, 128], mybir.dt.bfloat16, tag="tsh")
            for g in range(4):
                nc.sync.dma_start(tsh_t[:, g, :, :], tsh_d[:, g, :, :])
            cbf = wpool.tile([128, 8 * 128 + 32], mybir.dt.bfloat16)
            nc.sync.dma_start(cbf[:], cbf_d[:])
            w2bf = [cbf[:, 128 * l:128 * (l + 1)] for l in range(GL)]
            sel8 = cbf[:, 1024:1040]
            bones1 = cbf[:, 1040:1056]
            cdr = wpool.tile([128, 8, 2, 128], mybir.dt.float8e5, tag="cdr")
            nc.sync.dma_start(cdr[:], cdr_d[:])
            wdr = [cdr[:, l, :, :] for l in range(GL)]
            csm = wpool.tile([16, 9 * 128], mybir.dt.bfloat16, tag="csm")
            nc.sync.dma_start(csm[:], csm_d[:])
            negsa8 = csm[:, 0:128]
            negselb8 = [csm[:, 128 * (1 + l):128 * (2 + l)] for l in range(GL)]

            # ---- MT = Tsh^T @ x^T : [(o,k), a] in 4 chunks of 128 partitions ----
            mt = []      # bf16 working copy
            mtf32 = []   # fp32 upcast of the bf16-rounded values (ts scalar operand)
            nmt32 = []   # negated fp32 (ACT Relu bias)
            for g in range(4):
                pm = pmt.tile([128, N], mybir.dt.float32)
                for it in range(8):
                    nc.tensor.matmul(
                        pm[:],
                        tsh_t[:, g, it, :],
                        xt_t[:, it, :],
                        start=(it == 0),
                        stop=(it == 7),
                    )
                mt_g = mtpool.tile([128, N], mybir.dt.bfloat16, tag=f"mt{g}")
                nc.vector.tensor_copy(mt_g[:], pm[:])
                mt32_g = mtpool.tile([128, N], mybir.dt.float32, tag=f"mt32{g}")
                nc.scalar.copy(mt32_g[:], mt_g[:])
                nm_g = mtpool.tile([128, N], mybir.dt.float32, tag=f"nmt32{g}")
                nc.gpsimd.tensor_scalar(
                    nm_g[:], mt_g[:], -1.0, None, mybir.AluOpType.mult,
                )
                mt.append(mt_g)
                mtf32.append(mt32_g)
                nmt32.append(nm_g)

            # ---- ST[o, a] = sum_k MT ----
            st_ps = psmall.tile([16, N], mybir.dt.float32, tag="st_ps")
            for g in range(4):
                nc.tensor.matmul(
                    st_ps[:], bones1, mt[g][:], start=(g == 0), stop=(g == 3)
                )
            st_bf = mtpool.tile([16, N], mybir.dt.bfloat16, tag="st_bf")
            nc.vector.tensor_copy(st_bf[:], st_ps[:])

            # ---- bias tile: negsb8[16*l + o, grp] = -ST[o, 8*grp + l] ----
            nsb_ps = psmall.tile([128, NGRP], mybir.dt.float32, tag="nsb_ps")
            for l in range(GL):
                nc.tensor.matmul(
                    nsb_ps[:], negselb8[l], st_bf[:, l::GL],
                    start=(l == 0), stop=(l == GL - 1),
                )
            negsb8 = obpool.tile([128, NGRP], mybir.dt.float32, tag="negsb8")
            nc.vector.tensor_copy(negsb8[:], nsb_ps[:])

            ob_acc = obpool.tile([128, NGRP], mybir.dt.float32)
            obt_acc = obpool.tile([O, N - BW], mybir.dt.float32, tag="obt_acc")
            nc.vector.memset(obt_acc[:], 0.0)

            psmall_cm.__exit__(None, None, None)
            setup_psum_cm.__exit__(None, None, None)
            pnorm_cm = tc.tile_pool(
                name="psum_norm", bufs=PNORM_BUFS, space=bass.MemorySpace.PSUM,
            )
            pnorm = pnorm_cm.__enter__()
            obt_cm = tc.tile_pool(name="psum_obt", bufs=OBT_BUFS, space=bass.MemorySpace.PSUM)
            obt_pool = obt_cm.__enter__()

            # ---- pairwise: 8 triangle blocks x 2 group-pairs of 2x8 b's ----
            # Pair emission order: optionally interleave big-FD and small-FD
            # blocks so the latency-bound tail overlaps dense work.
            pair_order = [(blk, jp) for blk in range(NBLK) for jp in range(2)]
            if INTERLEAVE:
                first = [(blk, jp) for blk in range(NBLK // 2) for jp in range(2)]
                second = [(blk, jp) for blk in range(NBLK // 2, NBLK) for jp in range(2)][::-1]
                pair_order = [x for p in zip(first, second) for x in p]

            obt_state = {}   # blk -> (obt_ps tile, pairs_done)

            import contextlib
            rep_ctx = tc.For_i(0, reps, 1) if reps > 1 else contextlib.nullcontext()
            with rep_ctx:
                for blk, jp in pair_order:
                    a0 = BW * blk
                    FD = N - a0
                    pat = sched[blk]
                    if blk not in obt_state:
                        obt_ps = None
                        if blk < NBLK - 1:
                            obt_ps = obt_pool.tile([O, FD - BW], mybir.dt.float32, tag="obt")
                        obt_state[blk] = [obt_ps, 0]
                    obt_ps = obt_state[blk][0]

                    grp0 = 4 * blk + 2 * jp          # first group of the pair
                    bb = [a0 + GL * (2 * jp + h) for h in range(2)]  # b base per half
                    nt2 = pnorm.tile([128, 2 * FD], mybir.dt.float32, tag="nt")
                    first_mm = True
                    if ABLATE in ("nots", "skel"):
                        ad_fix = adpool.tile([128, 2, N], mybir.dt.bfloat16, tag="adfix")
                        ad8_fix = ad8pool.tile([128, 2, N], mybir.dt.float8e5, tag="ad8fix")
                        nc.vector.memset(ad_fix[:], 1.0)
                        nc.vector.memset(ad8_fix[:], 1.0)
                    for l in range(GL):
                        for pair in range(2):
                            eng = pat[2 * l + pair]
                            g0, g1 = 2 * pair, 2 * pair + 1
                            if eng == "D":
                                # one [128, 2(b), FD] tile per chunk, both groups
                                for g in (g0, g1):
                                    if ABLATE in ("nots", "skel"):
                                        ad = ad_fix
                                    else:
                                        ad = adpool.tile([128, 2, N], mybir.dt.bfloat16, tag="adD")
                                        for h in range(2):
                                            if SPLIT_TS and FD >= 64:
                                                hf = FD // 2
                                                nc.vector.tensor_scalar(
                                                    ad[:, h, :hf], mt[g][:, a0:a0 + hf],
                                                    mtf32[g][:, bb[h] + l:bb[h] + l + 1], 0.0,
                                                    mybir.AluOpType.subtract, mybir.AluOpType.max,
                                                )
                                                nc.vector.tensor_scalar(
                                                    ad[:, h, hf:FD], mt[g][:, a0 + hf:],
                                                    mtf32[g][:, bb[h] + l:bb[h] + l + 1], 0.0,
                                                    mybir.AluOpType.subtract, mybir.AluOpType.max,
                                                )
                                            else:
                                                nc.vector.tensor_scalar(
                                                    ad[:, h, :FD], mt[g][:, a0:],
                                                    mtf32[g][:, bb[h] + l:bb[h] + l + 1], 0.0,
                                                    mybir.AluOpType.subtract, mybir.AluOpType.max,
                                                )
                                    if ABLATE in ("nomm", "skel"):
                                        first_mm = False
                                        continue
                                    if PAIR_MM and FD == N:
                                        nc.tensor.matmul(
                                            nt2[:, 0:2 * FD], w2bf[l], ad[:, :, :],
                                            start=first_mm, stop=False,
                                            skip_group_check=True,
                                        )
                                    elif PAIR_MM:
                                        nc.tensor.matmul(
                                            nt2[:, 0:2 * FD], w2bf[l], ad[:, :, :FD],
                                            start=first_mm, stop=False,
                                            skip_group_check=True,
                                        )
                                    else:
                                        for h in range(2):
                                            nc.tensor.matmul(
                                                nt2[:, h * FD:(h + 1) * FD], w2bf[l], ad[:, h, :FD],
                                                start=first_mm, stop=False,
                                                skip_group_check=True,
                                            )
                                            first_mm = False
                                    first_mm = False
                            else:
                                for h in range(2):
                                    b = bb[h] + l
                                    if ABLATE in ("nots", "skel"):
                                        ad2 = ad8_fix
                                    else:
                                        ad2 = ad8pool.tile([128, 2, N], mybir.dt.float8e5, tag="ad8")
                                        if eng == "P":
                                            for i, g in enumerate((g0, g1)):
                                                nc.gpsimd.tensor_scalar(
                                                    ad2[:, i, :FD], mt[g][:, a0:],
                                                    mtf32[g][:, b:b + 1], 0.0,
                                                    mybir.AluOpType.subtract, mybir.AluOpType.max,
                                                )
                                        else:
                                            for i, g in enumerate((g0, g1)):
                                                nc.scalar.activation(
                                                    ad2[:, i, :FD], mt[g][:, a0:],
                                                    mybir.ActivationFunctionType.Relu,
                                                    bias=nmt32[g][:, b:b + 1],
                                                )
                                    if ABLATE in ("nomm", "skel"):
                                        continue
                                    nc.tensor.matmul(
                                        nt2[:, h * FD:(h + 1) * FD], wdr[l], ad2[:, :, :FD],
                                        start=False, stop=False,
                                        perf_mode=mybir.MatmulPerfMode.DoubleRow,
                                        skip_group_check=True,
                                    )
                    # -S_a terms close each half's accumulation
                    for h in range(2):
                        nc.tensor.matmul(
                            nt2[:, h * FD:(h + 1) * FD], negsa8, st_bf[:, a0:],
                            start=False, stop=True, skip_group_check=True,
                        )
                    for h in range(2):
                        grp = grp0 + h
                        e = epool.tile([128, N], mybir.dt.bfloat16, tag="e")
                        nc.scalar.activation(
                            e[:, :FD], nt2[:, h * FD:(h + 1) * FD],
                            mybir.ActivationFunctionType.Exp,
                            scale=-1.0, bias=negsb8[:, grp:grp + 1],
                            accum_out=ob_acc[:, grp:grp + 1],
                        )
                        if obt_ps is not None:
                            nc.tensor.matmul(
                                obt_ps[:], sel8, e[:, BW:FD],
                                start=(obt_state[blk][1] == 0 and h == 0),
                                stop=(obt_state[blk][1] == 1 and h == 1),
                                skip_group_check=True,
                            )
                    obt_state[blk][1] += 1
                    if obt_state[blk][1] == 2 and obt_ps is not None:
                        nc.vector.tensor_tensor(
                            obt_acc[:, a0:], obt_acc[:, a0:], obt_ps[:],
                            mybir.AluOpType.add,
                        )
                    if obt_state[blk][1] == 2:
                        del obt_state[blk]

            obt_cm.__exit__(None, None, None)
            pnorm_cm.__exit__(None, None, None)
            ob_final = obpool.tile([128, NGRP], mybir.dt.float32)
            nc.vector.tensor_scalar_add(ob_final[:], ob_acc[:], -1.0)
            nc.sync.dma_start(ob_d[:], ob_final[:])
            nc.sync.dma_start(ob2_d[:], obt_acc[:])

    nc.compile()
    return nc


def host_prep_shared(x):
    xt = np.ascontiguousarray(
        x.T.reshape(8, 128, N).transpose(1, 0, 2)
    ).astype(BF16)                                       # [128, 8, 256]
    cbf = np.zeros((128, 8 * 128 + 32), dtype=BF16)
    for l in range(GL):
        for p in range(128):
            cbf[p, 128 * l + 16 * l + p // 8] = 2.0      # w2bf[l]
    for p in range(128):
        cbf[p, 1024 + (p % 16)] = 1.0                    # sel8
        cbf[p, 1040 + p // 8] = 1.0                      # bones1
    csm = np.zeros((16, 9 * 128), dtype=BF16)
    for o in range(16):
        for l in range(GL):
            csm[o, 16 * l + o] = -1.0                    # negsa8
            csm[o, 128 * (1 + l) + 16 * l + o] = -1.0    # negselb8[l]
    cdr = np.zeros((128, 8, 2, 128), dtype=F8E5)
    for l in range(GL):
        for p in range(128):
            cdr[p, l, :, 16 * l + p // 8] = 2.0          # wdr[l]
    return xt, cbf, csm, cdr


def pack_tsh(T_core):
    """T_core [IN_F, O, K] -> [128, 4, 8, 128]: [p, g, it, m] = Tsh[it*128+p, g*128+m]
    with Tsh col m = o*8 + k_l, k = 8g + k_l."""
    tsh = np.ascontiguousarray(
        T_core.reshape(IN_F, O, 4, 8).transpose(0, 2, 1, 3).reshape(IN_F, 4, 128)
    )                                                    # [i, g, m]
    return np.ascontiguousarray(
        tsh.reshape(8, 128, 4, 128).transpose(1, 2, 0, 3)
    ).astype(BF16)                                       # [p, g, it, m]


def make_in_maps(x, T):
    xt, cbf, csm, cdr = host_prep_shared(x)
    in_maps = []
    for c in range(NCORES):
        tsh = pack_tsh(T[:, c * O:(c + 1) * O, :])
        in_maps.append({"xt": xt, "tsh": tsh, "cbf": cbf, "csm": csm, "cdr": cdr})
    return in_maps


def unscramble(ob_raw, ob2):
    """ob_raw [128, 32], ob2 [16, 224] -> ob [256, 16].

    b = 8*grp + l; ob_raw row = 16*l + o, col = grp.
    ob2[o, a'-32] holds the transposed-triangle contributions for a' >= 32.
    """
    a = np.asarray(ob_raw).reshape(GL, O, NGRP)        # [l, o, grp]
    ob = a.transpose(2, 0, 1).reshape(N, O).copy()     # [b, o]
    ob[BW:, :] += np.asarray(ob2).T
    return ob


_NC_CACHE = None


def kernel(x, T):
    global _NC_CACHE
    x = np.asarray(x, dtype=np.float32)
    T = np.asarray(T, dtype=np.float32)
    assert x.shape == (N, IN_F) and T.shape == (IN_F, OUT_F, K)

    if _NC_CACHE is None:
        _NC_CACHE = build_core_program()
    nc = _NC_CACHE

    in_maps = make_in_maps(x, T)
    res = run_bass_kernel_spmd(nc, in_maps, core_ids=list(range(NCORES)))

    cores = [unscramble(r["ob"], r["ob2"]) for r in res.results]
    ob = np.concatenate(cores, axis=1).astype(np.float32)

    out = np.empty((N, IN_F + OUT_F), dtype=np.float32)
    out[:, :IN_F] = x
    out[:, IN_F:] = ob
    return out
